# revision 32
# baseline (speedup 1.0000x reference)
"""Causal self-attention (RoPE + qk-RMS-norm) Trainium2 kernel.

Sharding: 8 cores = 2 batches x 4 head-groups (tensor-parallel over heads,
data-parallel over batch). Each core computes its head-group's attention and
a row-parallel partial of the output projection; the host sums the 4
per-group partials per batch (the all-reduce of row-parallel sharding).

Per-core layout: Q.T/K.T computed directly in [d, t] (no transposes),
V in [t, d]. Transposed flash attention: S.T = K @ Q.T so P.T feeds the
PV matmul directly; softmax has no max-subtraction (RMS-normed scores are
bounded by sqrt(D)); column sums via ones-matmul; 1/sum deferred to Y.T.
Matmuls run in float32r (full PE rate for N>=256). Tokens are processed in
two causal passes (halves of T) to fit SBUF.
"""

import functools

import numpy as np

B, T, C, H, D = 2, 2048, 1280, 10, 128
EPS = 1e-5
NHL = 3  # head slots per core (padded)
N_CORES = 8
NHALF = 2  # causal passes over T
# per-batch head groups (4th group padded with zero heads)
GROUPS = [[0, 1, 2], [3, 4, 5], [6, 7, 8], [9]]


def _emit(nc, tile, mybir, T, C, D, NHL, eps):
    F32 = mybir.dt.float32
    F32R = mybir.dt.float32r
    BF16 = mybir.dt.bfloat16
    ActF = mybir.ActivationFunctionType
    CCH = C // 128  # contraction chunks
    TBN = T // 128  # 128-token blocks
    T2 = T // NHALF  # tokens per pass
    TB2 = T2 // 128
    Q42 = T2 // 512  # q supertiles per pass
    HD = NHL * D
    couts = []
    off = 0
    while off < C:
        w = min(512, C - off)
        couts.append((off, w))
        off += w

    xt = nc.dram_tensor("xt", [C, T], BF16, kind="ExternalInput")
    wqt = nc.dram_tensor("wqt", [C, HD], BF16, kind="ExternalInput")
    wkt = nc.dram_tensor("wkt", [C, HD], BF16, kind="ExternalInput")
    wvt = nc.dram_tensor("wvt", [C, HD], BF16, kind="ExternalInput")
    wpt = nc.dram_tensor("wpt", [HD, C], BF16, kind="ExternalInput")
    cs = nc.dram_tensor("cs", [D, T], BF16, kind="ExternalInput")
    sc = nc.dram_tensor("sc", [D, T], BF16, kind="ExternalInput")
    out = nc.dram_tensor("out", [T, C], F32, kind="ExternalOutput")

    from contextlib import ExitStack

    with ExitStack() as ctx:
        ctx.enter_context(nc.allow_low_precision(reason="fp32r matmul operands"))
        tc = ctx.enter_context(tile.TileContext(nc))
        pool = lambda n, b, **kw: ctx.enter_context(tc.tile_pool(name=n, bufs=b, **kw))
        drp = pool("dr", 2, space="DRAM")
        per = pool("persist", 1)
        wvp = pool("wv", 1)
        wqkp = pool("wqk", 1)
        wptp = pool("wpt", 1)
        xtp = pool("xt", 1)
        qtp = pool("qt", 2)
        qsp = pool("qs", 1)
        ytp = pool("yt", 1)
        tmp = pool("tmp", 2)
        sqp = pool("sqp", 1)
        ptp = pool("ptp", 3)
        rows = pool("rows", 2)
        oev = pool("oev", 2)
        psmm = pool("psmm", 2, space="PSUM")
        psacc = pool("psacc", 2, space="PSUM")
        psrow = pool("psrow", 2, space="PSUM")

        # ---- constants ----
        ones_f = per.tile([128, 128], F32, tag="onf")
        nc.vector.memset(ones_f[:], 1.0)
        zeros_f = per.tile([128, 384], BF16, tag="zf")
        nc.vector.memset(zeros_f[:], 0.0)
        ones_col = per.tile([128, 1], BF16, tag="onc")
        nc.scalar.copy(ones_col[:], ones_f[:, 0:1])
        ones_col32 = per.tile([128, 1], F32R, tag="onc32")
        nc.scalar.copy(ones_col32[:], ones_f[:, 0:1])
        ones_row = per.tile([1, 128], BF16, tag="onr")
        nc.scalar.copy(ones_row[:], ones_f[0:1, :])
        beps_col = per.tile([128, 1], F32, tag="bepsc")
        nc.vector.memset(beps_col[:], float(eps))
        # 0/1 mask: keep tq >= tk in [tk, tq] layout (upper incl diag)
        tri01 = per.tile([128, 128], BF16, tag="tri")
        nc.vector.memset(tri01[:], 1.0)
        nc.gpsimd.affine_select(
            out=tri01[:], in_=tri01[:],
            compare_op=mybir.AluOpType.is_ge,
            fill=0.0, base=0,
            pattern=[[1, 128]], channel_multiplier=-1,
        )
        # rope half-mix selectors, M padded to 128 with disjoint columns:
        # y[0:64] = MA.T@t1 (cols 64-127 zero), y[64:128] = MB.T@t2
        # (cols 0-63 zero); the two matmuls accumulate in PSUM.
        ma_f = oev.tile([128, 128], F32, tag="ot", name="ma_f")
        mb_f = oev.tile([128, 128], F32, tag="ot", name="mb_f")
        nc.vector.memset(ma_f[:], 0.0)
        nc.vector.memset(mb_f[:], 0.0)
        nc.gpsimd.affine_select(
            out=ma_f[:, 0:64], in_=ma_f[:, 0:64],
            compare_op=mybir.AluOpType.not_equal,
            fill=1.0, base=0, pattern=[[-1, 64]], channel_multiplier=1,
        )
        nc.gpsimd.affine_select(
            out=ma_f[:, 0:64], in_=ma_f[:, 0:64],
            compare_op=mybir.AluOpType.not_equal,
            fill=1.0, base=-64, pattern=[[-1, 64]], channel_multiplier=1,
        )
        nc.gpsimd.affine_select(
            out=mb_f[:, 64:128], in_=mb_f[:, 64:128],
            compare_op=mybir.AluOpType.not_equal,
            fill=-1.0, base=0, pattern=[[-1, 64]], channel_multiplier=1,
        )
        nc.gpsimd.affine_select(
            out=mb_f[:, 64:128], in_=mb_f[:, 64:128],
            compare_op=mybir.AluOpType.not_equal,
            fill=1.0, base=-64, pattern=[[-1, 64]], channel_multiplier=1,
        )
        ma = per.tile([128, 128], BF16, tag="ma")
        mb = per.tile([128, 128], BF16, tag="mb")
        nc.scalar.copy(ma[:], ma_f[:])
        nc.scalar.copy(mb[:], mb_f[:])

        # PE warm-up: dummy accumulating matmuls during the initial DMA ramp
        warm = nc.dram_tensor("warm", [1, 512], F32, kind="ExternalOutput")
        wrhs = per.tile([128, 512], BF16, tag="wrhs")
        for i in range(4):
            nc.scalar.copy(wrhs[:, i * 128 : (i + 1) * 128], ones_f[:])
        wps = psrow.tile([1, 512], F32, tag="row", name="warmps")
        NWARM = 32
        for i in range(NWARM):
            nc.tensor.matmul(
                wps[:], ones_col[:], wrhs[:], start=(i == 0), stop=(i == NWARM - 1)
            )
        wsb = rows.tile([1, 512], F32, tag="rw", name="warmsb")
        nc.vector.tensor_copy(wsb[:], wps[:])
        nc.sync.dma_start(warm[:], wsb[:])

        # V for all heads/all tokens: [tk-part, tb, h, d]
        v_t = per.tile([128, TBN, NHL, D], BF16, tag="v")
        # K.T per head, all tokens
        ktr = [per.tile([128, T], BF16, tag=f"ktr{h}", name=f"ktr{h}")
               for h in range(NHL)]
        rk_cols = [per.tile([128, TBN], F32, tag=f"rkc{h}", name=f"rkc{h}")
                   for h in range(NHL)]

        # V/Q/K-projection weights, all heads (resident); issue the loads
        # from different engines so descriptor generation runs in parallel
        wv = []
        wqr = []
        wkr = []
        for c in range(CCH):
            t = wvp.tile([128, HD], BF16, tag=f"wv{c}")
            nc.scalar.dma_start(t[:], wvt[c * 128 : (c + 1) * 128, :])
            wv.append(t)
            tq = wqkp.tile([128, HD], BF16, tag=f"wq{c}")
            nc.sync.dma_start(tq[:], wqt[c * 128 : (c + 1) * 128, :])
            wqr.append(tq)
            tk = wqkp.tile([128, HD], BF16, tag=f"wk{c}")
            nc.gpsimd.dma_start(tk[:], wkt[c * 128 : (c + 1) * 128, :])
            wkr.append(tk)
        # output-projection weights (resident)
        wp = {}
        for hh in range(NHL):
            for ci, (co, cw) in enumerate(couts):
                t = wptp.tile([128, cw], BF16, tag=f"wp{hh}_{ci}")
                nc.sync.dma_start(
                    t[:], wpt[hh * 128 : (hh + 1) * 128, co : co + cw]
                )
                wp[(hh, ci)] = t

        def emit_attention(hf, h, qtn, ytn):
            """Attention for head h over this pass's q supertiles.
            kb-outer (K/V stationary reuse); st/exp run one kb ahead of
            PV/colsum so the in-order PE queue never waits on ACT."""
            gq4s = [hf * Q42 + q4 for q4 in range(Q42)]
            yts = [psacc.tile([128, 512], F32, tag="acc", name=f"yt{q4}")
                   for q4 in range(Q42)]
            # P column-sum accumulator (DVE, f32r so the final ones-matmul
            # streams at full rate)
            pacc = sqp.tile([128, Q42 * 512], F32R, tag="pacc", bufs=2)
            kbmax = 4 * (gq4s[-1] + 1)
            LA = 2  # st/exp run this many kb steps ahead of PV
            pts = {}  # kb -> pair pt tile awaiting PV
            for kb in range(kbmax + LA):
                if kb < kbmax:
                    active = [q4 for q4 in range(Q42) if kb <= 4 * gq4s[q4] + 3]
                    st = psmm.tile([128, Q42 * 512], F32, tag="mm", name="st")
                    for q4 in active:
                        lsl = slice(q4 * 512, (q4 + 1) * 512)
                        nc.tensor.matmul(
                            st[:, lsl],
                            ktr[h][:, kb * 128 : (kb + 1) * 128],
                            qtn[:, lsl],
                            start=True, stop=True,
                        )
                    pt = ptp.tile([128, Q42 * 512], BF16, tag="pt")
                    # one exp over the contiguous valid span of all active q4s
                    q0 = active[0]
                    j0 = kb - 4 * gq4s[q0]
                    lo = q0 * 512 + (j0 * 128 if j0 > 0 else 0)
                    hi = (active[-1] + 1) * 512
                    nc.scalar.activation(
                        pt[:, lo:hi], st[:, lo:hi], ActF.Exp,
                        scale=rk_cols[h][:, kb : kb + 1],
                    )
                    if j0 > 0:
                        nc.scalar.copy(
                            pt[:, q0 * 512 : lo],
                            zeros_f[:, : j0 * 128],
                        )
                    if 0 <= j0 <= 3:
                        dg = slice(q0 * 512 + j0 * 128, q0 * 512 + (j0 + 1) * 128)
                        nc.vector.tensor_mul(pt[:, dg], pt[:, dg], tri01[:])
                    for q4 in active:
                        lsl = slice(q4 * 512, (q4 + 1) * 512)
                        if kb == 0:
                            nc.vector.tensor_copy(pacc[:, lsl], pt[:, lsl])
                        else:
                            nc.vector.tensor_add(
                                pacc[:, lsl],
                                pacc[:, lsl].bitcast(F32), pt[:, lsl],
                            )
                    pts[kb] = pt
                if kb >= LA:
                    pkb = kb - LA
                    pt = pts.pop(pkb)
                    for q4 in range(Q42):
                        gq4 = gq4s[q4]
                        last_kb = 4 * gq4 + 3
                        if pkb > last_kb:
                            continue
                        lsl = slice(q4 * 512, (q4 + 1) * 512)
                        nc.tensor.matmul(
                            yts[q4][:], v_t[:, pkb, h, :], pt[:, lsl],
                            start=(pkb == 0), stop=(pkb == last_kb),
                        )
            csrs = []
            for q4 in range(Q42):
                csum = psrow.tile([1, 512], F32, tag="row", name=f"cs{q4}")
                nc.tensor.matmul(
                    csum[:], ones_col32[:], pacc[:, q4 * 512 : (q4 + 1) * 512],
                    start=True, stop=True,
                )
                csr = rows.tile([1, 512], BF16, tag="rw", name="csr")
                nc.vector.tensor_copy(csr[:], csum[:])
                csrs.append(csr)

            def normalize(h=h, ytn=ytn, yts=yts, csrs=csrs):
                for q4 in range(Q42):
                    lsl = slice(q4 * 512, (q4 + 1) * 512)
                    bc = psmm.tile([128, 512], F32, tag="mm", name="bc")
                    nc.tensor.matmul(
                        bc[:], ones_row[:], csrs[q4][:], start=True, stop=True
                    )
                    bcs = tmp.tile([128, 512], F32, tag="bcs", name="bcs")
                    nc.vector.reciprocal_approx_fast(bcs[:], bc[:])
                    nc.vector.tensor_mul(ytn[:, h, lsl], yts[q4][:], bcs[:])

            return normalize

        pending = None  # deferred attention emitter for the previous head

        for hf in range(NHALF):
            toff = hf * T2
            # ---- per-pass cos/sin (stacked) ----
            cs_t = qtp.tile([D, T2], BF16, tag="cs", bufs=1)
            sc_t = qtp.tile([D, T2], BF16, tag="sc", bufs=1)
            nc.sync.dma_start(cs_t[:], cs[:, toff : toff + T2])
            nc.sync.dma_start(sc_t[:], sc[:, toff : toff + T2])
            # ---- load x.T chunks for this pass ----
            xc = []
            xeng = (nc.gpsimd, nc.sync, nc.scalar)
            for c in range(CCH):
                t = xtp.tile([128, T2], BF16, tag=f"x{c}")
                xeng[c % 3].dma_start(
                    t[:], xt[c * 128 : (c + 1) * 128, toff : toff + T2]
                )
                xc.append(t)

            # ---- V projection for this pass, all heads batched ----
            for tb in range(TB2):
                gtb = hf * TB2 + tb
                vp = psmm.tile([128, HD], F32, tag="mm", name="vp")
                for c in range(CCH):
                    nc.tensor.matmul(
                        vp[:],
                        xc[c][:, tb * 128 : (tb + 1) * 128],
                        wv[c][:],
                        start=(c == 0), stop=(c == CCH - 1),
                    )
                nc.vector.tensor_copy(v_t[:, gtb, :, :], vp[:])

            # Y.T for this pass (all heads)
            ytn = ytp.tile([128, NHL, T2], BF16, tag="ytn")

            for h in range(NHL):
                # ---- Q/K projections into PSUM, evicted early to SBUF ----
                hds = slice(h * D, (h + 1) * D)
                qsb = {}
                for isq, wt in enumerate((wqr, wkr)):
                    qps = psmm.tile([128, Q42 * 512], F32, tag="mm", name="qps")
                    for c in range(CCH):
                        for q4 in range(Q42):
                            nc.tensor.matmul(
                                qps[:, q4 * 512 : (q4 + 1) * 512],
                                wt[c][:, hds],
                                xc[c][:, q4 * 512 : (q4 + 1) * 512],
                                start=(c == 0), stop=(c == CCH - 1),
                            )
                    for q4 in range(Q42):
                        sb = qsp.tile([128, 512], BF16, tag=f"qs{isq}{q4}")
                        nc.vector.tensor_copy(
                            sb[:], qps[:, q4 * 512 : (q4 + 1) * 512]
                        )
                        qsb[(isq, q4)] = sb

                # ---- previous head's attention (dense PE block) ----
                if pending is not None:
                    norm_prev = pending()
                    pending = None
                else:
                    norm_prev = None

                qtn = qtp.tile([128, T2], BF16, tag="qtn")

                if norm_prev is not None:
                    norm_prev()

                # ---- rope + norm (chains overlap the attention above) ----
                for isq, (dst, doff) in enumerate(((qtn, 0), (ktr[h], toff))):
                    for q4 in range(Q42):
                        gsl = slice(toff + q4 * 512, toff + (q4 + 1) * 512)
                        dsl = slice(doff + q4 * 512, doff + (q4 + 1) * 512)
                        qp = qsb[(isq, q4)]
                        lsl4 = slice(q4 * 512, (q4 + 1) * 512)
                        t1 = tmp.tile([128, 512], BF16, tag="t1")
                        t2 = tmp.tile([128, 512], BF16, tag="t2")
                        nc.gpsimd.tensor_mul(t1[:], qp[:], cs_t[:, lsl4])
                        nc.gpsimd.tensor_mul(t2[:], qp[:], sc_t[:, lsl4])
                        rp = psmm.tile([128, 512], F32, tag="mm", name="rp")
                        nc.tensor.matmul(rp[:], ma[:], t1[:], start=True, stop=False)
                        nc.tensor.matmul(rp[:], mb[:], t2[:], start=False, stop=True)
                        nc.scalar.copy(dst[:, dsl], rp[:])
                    if isq == 0:
                        # q: rq = sqrt(1/ssq) (folds 1/sqrt(D); no eps -- pad
                        # heads get nonzero Wq host-side), applied to qtn
                        # columns via ones-outer broadcast
                        for q4 in range(Q42):
                            lsl = slice(q4 * 512, (q4 + 1) * 512)
                            sq = sqp.tile([128, 512], BF16, tag="sq")
                            nc.vector.tensor_mul(sq[:], qtn[:, lsl], qtn[:, lsl])
                            ssq = psrow.tile([1, 512], F32, tag="row", name="ssq")
                            nc.tensor.matmul(
                                ssq[:], ones_col[:], sq[:], start=True, stop=True
                            )
                            # rsqrt(ssq) = exp(-0.5*ln(ssq)); Ln+Exp share one
                            # act table so no table reloads against the big Exp
                            rw = rows.tile([1, 512], F32, tag="rw")
                            nc.scalar.activation(rw[:], ssq[:], ActF.Ln)
                            rwr = rows.tile([1, 512], BF16, tag="rwr", bufs=1)
                            nc.scalar.activation(rwr[:], rw[:], ActF.Exp, scale=-0.5)
                            bq = psmm.tile([128, 512], F32, tag="mm", name="bq")
                            nc.tensor.matmul(
                                bq[:], ones_row[:], rwr[:], start=True, stop=True
                            )
                            nc.vector.tensor_mul(qtn[:, lsl], qtn[:, lsl], bq[:])
                    else:
                        # k: rk = 1/sqrt(ssq/D + eps) as a row per q4, then one
                        # strided DMA transposes [1, T2] -> [128, TB2] columns
                        rkrow = rows.tile([1, T2], F32, tag="rkrow", bufs=1)
                        for q4 in range(Q42):
                            ksl = slice(toff + q4 * 512, toff + (q4 + 1) * 512)
                            lsl = slice(q4 * 512, (q4 + 1) * 512)
                            sk = sqp.tile([128, 512], BF16, tag="sq", name="sk")
                            nc.vector.tensor_mul(
                                sk[:], ktr[h][:, ksl], ktr[h][:, ksl]
                            )
                            ssk = psrow.tile([1, 512], F32, tag="row", name="ssk")
                            nc.tensor.matmul(
                                ssk[:], ones_col[:], sk[:], start=True, stop=True
                            )
                            # ln(ssk/D + eps), then exp(-0.5*..) = rsqrt
                            nc.scalar.activation(
                                rkrow[:, lsl], ssk[:], ActF.Ln,
                                scale=1.0 / D, bias=beps_col[0:1, :],
                            )
                        nc.scalar.activation(
                            rkrow[:], rkrow[:], ActF.Exp, scale=-0.5
                        )
                        # transpose [1, T2] -> [128, TB2] via a DRAM bounce
                        rkd = drp.tile([1, T2], F32, tag="rkd")
                        nc.sync.dma_start(rkd[:], rkrow[:])
                        nc.sync.dma_start(
                            rk_cols[h][:, hf * TB2 : (hf + 1) * TB2],
                            rkd[0:1, :].rearrange("a (j p) -> a p j", p=128),
                        )

                pending = (lambda hf=hf, h=h, qtn=qtn, ytn=ytn:
                           emit_attention(hf, h, qtn, ytn))

            # ---- last head's attention, then output projection ----
            if pending is not None:
                norm_last = pending()
                norm_last()
                pending = None
            for tb in range(TB2):
                for ci, (co, cw) in enumerate(couts):
                    op = psacc.tile([128, cw], F32, tag="acc", name="op")
                    for hh in range(NHL):
                        nc.tensor.matmul(
                            op[:],
                            ytn[:, hh, tb * 128 : (tb + 1) * 128],
                            wp[(hh, ci)][:],
                            start=(hh == 0), stop=(hh == NHL - 1),
                        )
                    ot = oev.tile([128, cw], F32, tag="ot")
                    if (tb * len(couts) + ci) % 2 == 0:
                        nc.vector.tensor_copy(ot[:], op[:])
                    else:
                        nc.scalar.copy(ot[:], op[:])
                    nc.sync.dma_start(
                        out[toff + tb * 128 : toff + (tb + 1) * 128, co : co + cw],
                        ot[:],
                    )
    return nc


@functools.lru_cache(maxsize=4)
def _build(T_=T, C_=C, D_=D, NHL_=NHL, eps=EPS):
    import concourse.bacc as bacc
    import concourse.tile as tile
    from concourse import mybir

    nc = bacc.Bacc("TRN2", target_bir_lowering=False)
    _emit(nc, tile, mybir, T_, C_, D_, NHL_, eps)
    nc.compile()
    return nc


def _shard(x, cos, sin, Wq, Wk, Wv, Wproj):
    """Build the 8 per-core input maps."""
    import ml_dtypes

    BF = ml_dtypes.bfloat16
    HD = NHL * D
    cosT = np.ascontiguousarray(cos[0, 0].T.astype(np.float32))  # [64, T]
    sinT = np.ascontiguousarray(sin[0, 0].T.astype(np.float32))
    cs = np.concatenate([cosT, sinT], axis=0).astype(BF)  # [128, T]
    sc = np.concatenate([sinT, cosT], axis=0).astype(BF)

    def head_rows(W, heads, pad=0.0):
        rows = np.full((HD, C), pad, np.float32)
        for i, h in enumerate(heads):
            rows[i * D : (i + 1) * D] = W[h * D : (h + 1) * D]
        return rows

    in_maps = []
    for b in range(B):
        xtb = np.ascontiguousarray(x[b].T).astype(BF)  # [C, T]
        for heads in GROUPS:
            wq = np.ascontiguousarray(head_rows(Wq, heads, pad=0.01).T).astype(BF)
            wk = np.ascontiguousarray(head_rows(Wk, heads).T).astype(BF)
            wv = np.ascontiguousarray(head_rows(Wv, heads).T).astype(BF)
            # Wproj columns for these heads, transposed: [HD, C]
            wp = np.zeros((HD, C), np.float32)
            for i, h in enumerate(heads):
                wp[i * D : (i + 1) * D] = Wproj[:, h * D : (h + 1) * D].T
            in_maps.append(
                {"xt": xtb, "wqt": wq, "wkt": wk, "wvt": wv,
                 "wpt": wp.astype(BF), "cs": cs, "sc": sc}
            )
    return in_maps


def _gather(results):
    y = np.zeros((B, T, C), np.float32)
    for b in range(B):
        for g in range(len(GROUPS)):
            y[b] += results[b * len(GROUPS) + g]["out"]
    return y


def _run(in_maps, trace=False):
    from concourse.bass_utils import run_bass_kernel_spmd

    nc = _build()
    return run_bass_kernel_spmd(
        nc, in_maps, core_ids=list(range(N_CORES)), trace=trace
    )


def kernel(x, cos, sin, Wq, Wk, Wv, Wproj):
    ins = _shard(
        np.asarray(x), np.asarray(cos), np.asarray(sin),
        np.asarray(Wq), np.asarray(Wk), np.asarray(Wv), np.asarray(Wproj),
    )
    res = _run(ins, trace=False)
    return _gather(res.results)


def run_traced(x, cos, sin, Wq, Wk, Wv, Wproj):
    ins = _shard(
        np.asarray(x), np.asarray(cos), np.asarray(sin),
        np.asarray(Wq), np.asarray(Wk), np.asarray(Wv), np.asarray(Wproj),
    )
    res = _run(ins, trace=True)
    return _gather(res.results), res

